# revision 15
# baseline (speedup 1.0000x reference)
"""Cosine-similarity attention (B=8, H=16, N=1024, D=64) on 8 trn2 NeuronCores.

Sharding: core c gets batch c (all 16 heads). No cross-core communication.

Per-core algorithm (per head pair A,B):
  - load q,k rows as [128p, 2h, 8blk, 64d] fp32 (partition p holds rows
    8p..8p+7; the induced row permutation is undone by the output DMA)
  - row sum-of-squares for q and k (GP square, DVE reduce), 1/norm =
    exp(-0.5*ln(ss)) on ACT (one combined ln+exp table set)
  - qn = q * rq (GP, bf16); k passes through RAW as bf16 (its 1/||k||
    is folded into the exp affine as a per-partition scale)
  - PE block-transpose qn/k -> qnT/kT [128(2x64d), 1024] bf16
  - row-tiled QK^T: S[m,i] for both heads concurrently (K=64 halves of
    the PE array) into [128, 1024] PSUM windows
  - exp split across engines:
      ACT wins:  e = exp(rk[m]*S + ln(s2))            (exact, scaled)
      DVE wins:  Schraudolph-in-bf16-bits, 2-term PWL:
                 i16 = trunc(S*rk*C + 16256+MU1); e = bits(i16)+bits(i16+D2)
                 (band +-1.4%, scale s2 matches the ACT wins; Z cancels s2)
  - AV: out[i,:64]|Z accumulated over m-chunks as e.T @ [v|ones], with
    M=64 col-tiling: two concurrent matmuls on PE col-groups (0,0)/(0,64)
  - normalize by 1/Z (DVE) into a staged [128, 8, 16, 64] output, one DMA out
"""

import math
import numpy as np
from contextlib import ExitStack

import concourse.bass as bass
import concourse.bacc as bacc
import concourse.mybir as mybir
import concourse.tile as tile
from concourse import bass_utils
from concourse.masks import make_identity

FP32 = mybir.dt.float32
BF16 = mybir.dt.bfloat16
I16 = mybir.dt.int16

N_CORES = 8
H = 16          # heads per core (= all heads; batch is sharded)
N = 1024
D = 64
NB = N // 128   # 8 row-blocks / m-chunks / i-blocks

# --- exp approximation constants (see module docstring) ---
C_ = 128.0 * math.log2(math.e)     # 184.6650
MU1 = 2.99                          # floor-rounding optimum
D2 = 60                             # second-term bits offset
LN_S2 = 0.918409                    # ln(2.50530): ACT scale-match bias

# --- engine assignment knobs ---
# win index w = mc*2 + ih (mc-major). DVE wins use the PWL path.
DVE_WINS = frozenset((2, 6, 10, 14))
I3_GP = frozenset((2, 10))          # i3 on GP for these wins (AV is a pair behind: slack)
AV_V2 = False                      # col-tiled AV was LDW-bound: slower


def emit_attention(ctx: ExitStack, tc: tile.TileContext, q, k, v, out, n_heads=H):
    nc = tc.nc
    mult = mybir.AluOpType.mult
    add = mybir.AluOpType.add
    AX = mybir.AxisListType.X
    Exp = mybir.ActivationFunctionType.Exp
    Ln = mybir.ActivationFunctionType.Ln

    def bcast(ap, n):
        # broadcast a [..., G] AP over a new innermost axis of length n
        return bass.AP(tensor=ap.tensor, offset=ap.offset, ap=[*ap.ap, [0, n]])

    singles = ctx.enter_context(tc.tile_pool(name="singles", bufs=1))
    qk_stage = ctx.enter_context(tc.tile_pool(name="qk_stage", bufs=2))
    v_stage = ctx.enter_context(tc.tile_pool(name="v_stage", bufs=2))
    small = ctx.enter_context(tc.tile_pool(name="small", bufs=2))
    qn_pool = ctx.enter_context(tc.tile_pool(name="qn_pool", bufs=2))
    tpose = ctx.enter_context(tc.tile_pool(name="tpose", bufs=2))
    expp = ctx.enter_context(tc.tile_pool(name="expp", bufs=2))
    pwl_pool = ctx.enter_context(tc.tile_pool(name="pwl_pool", bufs=2))
    zr_pool = ctx.enter_context(tc.tile_pool(name="zr_pool", bufs=4))
    out_pool = ctx.enter_context(tc.tile_pool(name="out_pool", bufs=2))

    tp_psum = ctx.enter_context(tc.tile_pool(name="tp_psum", bufs=2, space="PSUM"))
    qk_psum = ctx.enter_context(tc.tile_pool(name="qk_psum", bufs=3, space="PSUM"))

    identity = singles.tile([128, 128], BF16)
    make_identity(nc, identity)
    ln_s2c = singles.tile([128, 1], FP32)
    nc.gpsimd.memset(ln_s2c, LN_S2)

    out_r = out.rearrange("(p a) (h d) -> p a h d", a=NB, d=D)

    n_pairs = n_heads // 2

    def emit_prologue(t, pair):
        hA = 2 * pair
        # qk_raw index t (3rd dim): 0,1 = q headA/B ; 2,3 = k headA/B
        qk_raw = qk_stage.tile([128, NB, 4, D], FP32, tag="qk_raw")
        sq = small.tile([128, NB, 4, D], FP32, tag="sq")
        ss = small.tile([128, NB, 4], FP32, tag="ss")
        lns = small.tile([128, NB, 4], FP32, tag="lns")
        t["rr"] = rr = small.tile([128, NB, 4], FP32, tag="rr", name="rr")
        t["rk_c"] = rk_c = small.tile([128, NB, 2], FP32, tag="rk_c", name="rk_c")
        qn_all = qn_pool.tile([128, NB, 4, D], BF16, tag="qn_all")
        vraw = v_stage.tile([128, 2, NB, D], FP32, tag="vraw")
        for h0 in (0, NB // 2):
            sl = slice(h0, h0 + NB // 2)
            for ih in range(2):
                nc.sync.dma_start(
                    out=qk_raw[:, sl, ih],
                    in_=q[hA + ih].rearrange("(p a) d -> p a d", a=NB)[:, sl],
                )
                nc.sync.dma_start(
                    out=qk_raw[:, sl, 2 + ih],
                    in_=k[hA + ih].rearrange("(p a) d -> p a d", a=NB)[:, sl],
                )
            # ss = sum_d x^2 ; rr = exp(-0.5 ln(ss)) = 1/||x||
            nc.gpsimd.tensor_tensor(sq[:, sl], qk_raw[:, sl], qk_raw[:, sl], op=mult)
            nc.vector.reduce_sum(ss[:, sl], sq[:, sl], axis=AX)
            nc.scalar.activation(lns[:, sl], ss[:, sl], Ln)
            nc.scalar.activation(rr[:, sl], lns[:, sl], Exp, scale=-0.5)
            # q normalized to bf16 on GP; k passes through raw (rk folded
            # into the exp affine)
            nc.gpsimd.tensor_tensor(
                qn_all[:, sl, 0:2], qk_raw[:, sl, 0:2], bcast(rr[:, sl, 0:2], D), op=mult
            )
            nc.gpsimd.tensor_copy(qn_all[:, sl, 2:4], qk_raw[:, sl, 2:4])
            # rk_c = rr_k * C_ for the DVE-PWL affine (per-partition scalars)
            nc.vector.tensor_scalar(
                rk_c[:, sl], rr[:, sl, 2:4], float(C_), None, op0=mult
            )
        for ih in range(2):
            nc.sync.dma_start(
                out=vraw[:, ih],
                in_=v[hA + ih].rearrange("(p a) d -> p a d", a=NB),
            )
        # v|ones in bf16: [128, 2, NB, 65]
        t["vb"] = vb = v_stage.tile([128, 2, NB, D + 1], BF16, tag="vb", name="vb")
        nc.gpsimd.memset(vb[:, :, :, D : D + 1], 1.0)
        nc.gpsimd.tensor_copy(vb[:, :, :, 0:D], vraw)

        # transpose -> qnT/kT [128(=A|B stacked 64d), 1024] bf16
        t["qnT"] = qnT = tpose.tile([128, N], BF16, tag="qnT", name="qnT")
        t["knT"] = knT = tpose.tile([128, N], BF16, tag="knT", name="knT")
        for t0, dstT in ((0, qnT), (2, knT)):
            for ag in range(0, NB, 4):
                tp = tp_psum.tile([128, 4, 128], BF16, tag="tp")
                for j in range(4):
                    nc.tensor.transpose(
                        tp[:, j], qn_all[:, ag + j, t0 : t0 + 2, :], identity
                    )
                nc.vector.tensor_copy(dstT[:, ag * 128 : (ag + 4) * 128], tp)
        t["eAB"] = expp.tile([128, NB, 2, N], BF16, tag="eAB", name="eAB")
        t["out_pair"] = out_pool.tile([128, NB, 2, D], FP32, tag="out_pair", name="out_pair")
        t["hA"] = hA

    def emit_qk_exp_win(t, mc, ih):
        qnT, knT, rr, rk_c, eAB = t["qnT"], t["knT"], t["rr"], t["rk_c"], t["eAB"]
        win = qk_psum.tile([128, 1024], FP32, tag="win")
        for icc in range(2):
            nc.tensor.matmul(
                win[:, icc * 512 : (icc + 1) * 512],
                lhsT=knT[ih * 64 : (ih + 1) * 64, mc * 128 : (mc + 1) * 128],
                rhs=qnT[ih * 64 : (ih + 1) * 64, icc * 512 : (icc + 1) * 512],
                start=True,
                stop=True,
                tile_position=(ih * 64, 0),
            )
        w = mc * 2 + ih
        if w in DVE_WINS:
            # Schraudolph 2-term: e = bits(i16) + bits(i16 + D2)
            i16 = pwl_pool.tile([128, N], I16, tag="i16")
            j16 = pwl_pool.tile([128, N], I16, tag="j16")
            nc.vector.tensor_scalar(
                i16, win, rk_c[:, mc, ih : ih + 1], float(16256.0 + MU1),
                op0=mult, op1=add,
            )
            nc.vector.tensor_scalar(j16, i16, float(D2), None, op0=add)
            eng = nc.gpsimd if w in I3_GP else nc.vector
            eng.tensor_tensor(
                eAB[:, mc, ih], i16.bitcast(BF16), j16.bitcast(BF16), op=add
            )
        else:
            nc.scalar.activation(
                eAB[:, mc, ih], win, Exp,
                bias=ln_s2c, scale=rr[:, mc, 2 + ih : 3 + ih],
            )

    def emit_av_chunk(t, ih, bg, j):
        # one j-chunk: 8 accumulating matmuls into acc[:, j]
        eAB, vb = t["eAB"], t["vb"]
        if j == 0:
            t["acc"] = tp_psum.tile([128, 4, D + 1], FP32, tag="tp", name="acc")
        acc = t["acc"]
        for mc in range(NB):
            nc.tensor.matmul(
                acc[:, j],
                lhsT=eAB[:, mc, ih, (bg + j) * 128 : (bg + j + 1) * 128],
                rhs=vb[:, ih, mc, :],
                start=(mc == 0),
                stop=(mc == NB - 1),
            )

    def emit_av_norm(t, ih, bg):
        acc, out_pair = t["acc"], t["out_pair"]
        zr = zr_pool.tile([128, 4], FP32, tag="zr")
        nc.vector.reciprocal(zr, acc[:, :, D])
        nc.vector.tensor_tensor(
            out_pair[:, bg : bg + 4, ih],
            acc[:, :, 0:D],
            bcast(zr, D),
            op=mult,
        )

    def emit_av_block(t, ih, bg):
        for j in range(4):
            emit_av_chunk(t, ih, bg, j)
        emit_av_norm(t, ih, bg)

    def emit_av_tail(t):
        # all 4 AV blocks + output DMA for a pair
        for ih in range(2):
            for bg in range(0, NB, 4):
                emit_av_block(t, ih, bg)
        nc.sync.dma_start(out=out_r[:, :, t["hA"] : t["hA"] + 2, :], in_=t["out_pair"])

    # software pipeline: AV of pair p-1 interleaved into QK/exp of pair p,
    # so the PE chews AV matmuls while ACT/DVE consume exp windows.
    AV_BLOCKS = [(ih, bg) for ih in range(2) for bg in range(0, NB, 4)]
    prev = None
    for pair in range(n_pairs):
        cur = {}
        emit_prologue(cur, pair)
        wi = 0
        for mc in range(NB):
            for ih in range(2):
                emit_qk_exp_win(cur, mc, ih)
                if prev is not None:
                    bih, bbg = AV_BLOCKS[wi // 4]
                    emit_av_chunk(prev, bih, bbg, wi % 4)
                    if wi % 4 == 3:
                        emit_av_norm(prev, bih, bbg)
                wi += 1
        if prev is not None:
            nc.sync.dma_start(
                out=out_r[:, :, prev["hA"] : prev["hA"] + 2, :], in_=prev["out_pair"]
            )
        prev = cur
    emit_av_tail(prev)


class _Bacc(bacc.Bacc):
    """Bacc whose act-table pass only sees the combined ln+exp set, so Ln and
    Exp activations share one table load instead of thrashing between the
    single-function sets (~2.7us per reload on ACT)."""

    def insert_act_table_loads(self):
        import bass_rust as _bass_rust
        from concourse.hw_specs import get_activation_tables

        has_activation = any(
            isinstance(i, mybir.InstActivation)
            for b in self.main_func.blocks
            for i in b.instructions
        )
        if not has_activation:
            return
        tables = [
            (name, set() if name in ("exp_and_others", "natural_log", "exp_and_friends") else fns)
            for name, fns in get_activation_tables(self.m.arch).items()
        ]
        _bass_rust.insert_act_table_loads(self, tables)


def build_program(n_heads=H, num_devices=N_CORES, loop_iters=1):
    nc = _Bacc(
        "TRN2",
        target_bir_lowering=False,
        debug=False,
        enable_asserts=False,
        num_devices=num_devices,
    )
    qd = nc.dram_tensor("q", [n_heads, N, D], FP32, kind="ExternalInput").ap()
    kd = nc.dram_tensor("k", [n_heads, N, D], FP32, kind="ExternalInput").ap()
    vd = nc.dram_tensor("v", [n_heads, N, D], FP32, kind="ExternalInput").ap()
    od = nc.dram_tensor("out", [N, n_heads * D], FP32, kind="ExternalOutput").ap()
    with tile.TileContext(nc) as tc:
        with ExitStack() as ctx:
            if loop_iters > 1:
                with tc.For_i(0, loop_iters, 1):
                    with ExitStack() as ictx:
                        emit_attention(ictx, tc, qd, kd, vd, od, n_heads=n_heads)
            else:
                emit_attention(ctx, tc, qd, kd, vd, od, n_heads=n_heads)
    nc.compile()
    return nc


_PROGRAM = None


def kernel(q: np.ndarray, k: np.ndarray, v: np.ndarray, _trace=False, _trace_kwargs=None):
    """Full inputs [8, 16, 1024, 64] fp32 -> full output [8, 1024, 1024] fp32."""
    global _PROGRAM
    if _PROGRAM is None:
        _PROGRAM = build_program()
    nc = _PROGRAM

    from concourse.bass_interp import get_hw_module

    in_maps = [
        {
            "q": np.ascontiguousarray(np.asarray(q)[c], dtype=np.float32),
            "k": np.ascontiguousarray(np.asarray(k)[c], dtype=np.float32),
            "v": np.ascontiguousarray(np.asarray(v)[c], dtype=np.float32),
        }
        for c in range(N_CORES)
    ]
    old_m = nc.m
    nc.m = get_hw_module(nc.m)
    try:
        res = bass_utils.run_bass_kernel_spmd(
            nc,
            in_maps,
            core_ids=list(range(N_CORES)),
            trace=_trace,
            **(_trace_kwargs or {}),
        )
    finally:
        nc.m = old_m
    out = np.stack([res.results[c]["out"] for c in range(N_CORES)])
    if _trace:
        kernel.last_results = res
    return out
